# revision 1
# baseline (speedup 1.0000x reference)
"""Cross-attention Trainium2 kernel, SPMD over 8 NeuronCores.

Sharding: core c = b*4 + g handles batch b (of 2) and head-group g (of 4,
4 heads each) — data parallel on B, Megatron tensor parallel on heads:
W_qkv column-sliced, W_proj row-sliced, partial outputs summed on host.

Per-core dataflow (all matmuls bf16 operands, fp32 PSUM accumulation):
  - host pre-transposes x/context to [D, L] and pre-casts weights to bf16
  - Q^T[n,tok] = Wq.T @ x^T      (lhsT=Wq tile, rhs=x^T tile, + bias via ACT)
  - K^T[n,key] = Wk.T @ c^T      (same)
  - V[key,n]   = c @ Wv          (lhsT=c^T tile, rhs=Wv, bias via K=1 matmul)
  - S^T[key,tok] = K_h @ Q_h^T   (one matmul per tile, contraction=head_dim)
  - P^T = exp(scale * S^T)       (ACT, no max subtraction: scores ~ N(0,1))
  - O[tok,hd]+denom = P @ [V|1]  (lhsT=P^T slice, rhs=V_aug, ones column
                                  gives the softmax denominator in col 128)
  - normalize rows by 1/denom    (DVE reciprocal + per-partition scalar mul)
  - O^T via PE transpose
  - out[tok,dout] = O @ Wp       (lhsT=O^T tile, rhs=Wp tile)
"""

import numpy as np
import ml_dtypes

import concourse.bass as bass
import concourse.bacc as bacc
import concourse.mybir as mybir
from concourse.bass import ts
from concourse.masks import make_identity
from concourse.tile import TileContext

DIM = 2048
NUM_HEADS = 16
HEAD_DIM = 128
B, L = 2, 2048
GPB = 4                  # head-groups per batch (cores per batch)
HPC = NUM_HEADS // GPB   # heads per core = 4
NPC = HPC * HEAD_DIM     # per-core projection width = 512
N_CORES = 8

F32 = mybir.dt.float32
BF16 = mybir.dt.bfloat16
BF16_NP = ml_dtypes.bfloat16


def build_bass(dim=DIM, seq=L, hpc=HPC, hd=HEAD_DIM, repeat=1):
    """Build the per-core SPMD Bass program (parameterized for testing)."""
    npc = hpc * hd
    KT = dim // 128      # contraction tiles over model dim
    LT = seq // 128      # token 128-tiles
    TCH = seq // 512     # token 512-chunks
    scale = float(hd) ** -0.5
    Exp = mybir.ActivationFunctionType.Exp

    nc = bacc.Bacc()
    xT = nc.dram_tensor("xT", [dim, seq], BF16, kind="ExternalInput")
    cT = nc.dram_tensor("cT", [dim, seq], BF16, kind="ExternalInput")
    wq = nc.dram_tensor("wq", [dim, npc], BF16, kind="ExternalInput")
    wk = nc.dram_tensor("wk", [dim, npc], BF16, kind="ExternalInput")
    wv = nc.dram_tensor("wv", [dim, npc], BF16, kind="ExternalInput")
    bq = nc.dram_tensor("bq", [128, hpc], F32, kind="ExternalInput")
    bk = nc.dram_tensor("bk", [128, hpc], F32, kind="ExternalInput")
    bv = nc.dram_tensor("bv", [1, npc], BF16, kind="ExternalInput")
    wp = nc.dram_tensor("wp", [npc, dim], BF16, kind="ExternalInput")
    out = nc.dram_tensor("out", [seq, dim], F32, kind="ExternalOutput")

    with TileContext(nc) as tc:
        with (
            tc.tile_pool(name="psmm", bufs=4, space="PSUM") as psmm,
            tc.tile_pool(name="psov", bufs=2, space="PSUM") as psov,
            tc.tile_pool(name="pstr", bufs=2, space="PSUM") as pstr,
            tc.tile_pool(name="small", bufs=4) as small,
        ):
            for _rep in range(repeat):
                _build_body(
                    nc, tc, psmm, psov, pstr, small,
                    xT, cT, wq, wk, wv, bq, bk, bv, wp, out,
                    dim, seq, hpc, hd, npc, KT, LT, TCH, scale, Exp,
                )

    nc.compile()
    return nc


def _build_body(
    nc, tc, psmm, psov, pstr, small,
    xT, cT, wq, wk, wv, bq, bk, bv, wp, out,
    dim, seq, hpc, hd, npc, KT, LT, TCH, scale, Exp,
):
    with tc.tile_pool(name="res", bufs=1) as res:
        QT = res.tile([128, hpc, seq], BF16)    # [hd, h, tok]
        KTl = res.tile([128, hpc, seq], BF16)   # [hd, h, key]
        V = res.tile([128, LT, hpc, 130], BF16)  # [key, ktile, h, hd+1(ones)+pad]
        O = res.tile([128, LT, npc], BF16)      # [tok, ttile, h*hd]
        OT = res.tile([128, hpc, seq], BF16)    # [hd, h, tok] transposed O
        Wp_sb = res.tile([128, hpc, dim], BF16)
        bq_sb = res.tile([128, hpc], F32)
        bk_sb = res.tile([128, hpc], F32)
        bv_sb = res.tile([1, npc], BF16)
        ones_sb = res.tile([1, 128], BF16)
        ident = res.tile([128, 128], BF16)

        nc.vector.memset(ones_sb[:], 1.0)
        nc.vector.memset(V[:, :, :, 128:130], 1.0)
        make_identity(nc, ident[:])

        # ---- phase 1: projections ----
        with (
            tc.tile_pool(name="stream", bufs=2) as stream_pool,
            tc.tile_pool(name="wqkv", bufs=1) as wpool,
        ):
            Wq_sb = wpool.tile([128, KT, npc], BF16)
            Wk_sb = wpool.tile([128, KT, npc], BF16)
            Wv_sb = wpool.tile([128, KT, npc], BF16)
            wk_r = wk[:, :].rearrange("(kt p) n -> p kt n", p=128)
            wv_r = wv[:, :].rearrange("(kt p) n -> p kt n", p=128)
            wq_r = wq[:, :].rearrange("(kt p) n -> p kt n", p=128)

            def stream_chunk(src, t):
                st_tile = stream_pool.tile([128, KT, 512], BF16, tag="stream")
                src_r = src[:, ts(t, 512)].rearrange("(kt p) n -> p kt n", p=128)
                nc.sync.dma_start(st_tile[:], src_r[:, :, :])
                return st_tile

            # load order: Wk + first ctx chunk k-tiles interleaved so the
            # first matmul only waits on ~256KB, and the SP engine's
            # serial DMA-issue doesn't delay it
            cst0 = stream_pool.tile([128, KT, 512], BF16, tag="stream")
            cT0_r = cT[:, 0:512].rearrange("(kt p) n -> p kt n", p=128)
            for kt in range(KT):
                nc.sync.dma_start(Wk_sb[:, kt, :], wk_r[:, kt, :])
                nc.sync.dma_start(cst0[:, kt, :], cT0_r[:, kt, :])
            nc.sync.dma_start(bk_sb[:], bk[:, :])
            nc.sync.dma_start(bv_sb[:], bv[:, :])
            nc.sync.dma_start(Wv_sb[:], wv_r[:, :, :])
            nc.sync.dma_start(Wq_sb[:], wq_r[:, :, :])
            nc.sync.dma_start(bq_sb[:], bq[:, :])
            nc.sync.dma_start(
                Wp_sb[:], wp[:, :].rearrange("(h p) d -> p h d", p=128)
            )

            # context side first: K^T and V (attention waits on these)
            for t in range(TCH):
                cst = cst0 if t == 0 else stream_chunk(cT, t)
                for h in range(hpc):
                    ps = psmm.tile([128, 512], F32, tag="mm512")
                    for kt in range(KT):
                        nc.tensor.matmul(
                            ps[:],
                            Wk_sb[:, kt, ts(h, 128)],
                            cst[:, kt, :],
                            start=(kt == 0),
                            stop=(kt == KT - 1),
                        )
                    nc.scalar.add(KTl[:, h, ts(t, 512)], ps[:], bk_sb[:, h : h + 1])
                for j in range(4):
                    kt2 = t * 4 + j
                    ps = psmm.tile([128, 512], F32, tag="mm512")
                    for kt in range(KT):
                        nc.tensor.matmul(
                            ps[:, :npc],
                            cst[:, kt, ts(j, 128)],
                            Wv_sb[:, kt, :],
                            start=(kt == 0),
                            stop=False,
                        )
                    # bias row via K=1 matmul: ones.T @ bv accumulates b_v
                    nc.tensor.matmul(
                        ps[:, :npc],
                        ones_sb[0:1, :],
                        bv_sb[0:1, :],
                        start=False,
                        stop=True,
                    )
                    for h in range(hpc):
                        nc.vector.tensor_copy(
                            V[:, kt2, h, 0:128], ps[:, ts(h, 128)]
                        )

            for t in range(TCH):
                xst = stream_chunk(xT, t)
                for h in range(hpc):
                    ps = psmm.tile([128, 512], F32, tag="mm512")
                    for kt in range(KT):
                        nc.tensor.matmul(
                            ps[:],
                            Wq_sb[:, kt, ts(h, 128)],
                            xst[:, kt, :],
                            start=(kt == 0),
                            stop=(kt == KT - 1),
                        )
                    nc.scalar.add(QT[:, h, ts(t, 512)], ps[:], bq_sb[:, h : h + 1])

        # ---- phase 2+3+4 interleaved: attention, transpose, out-proj ----
        # t outer so a token group's O finishes after its 4 heads; its
        # transposes + projection matmuls then fill PE while ACT exps the
        # next iteration's scores.
        with tc.tile_pool(name="ptpool", bufs=3) as ppool:
            iters = [(t, h) for t in range(TCH) for h in range(hpc)]
            pts = {}

            def do_st(t, h):
                PT = ppool.tile([128, LT, 512], BF16, tag="pt")
                pts[(t, h)] = PT
                for kt2 in range(LT):
                    ps = psmm.tile([128, 512], F32, tag="mm512")
                    nc.tensor.matmul(
                        ps[:],
                        KTl[:, h, ts(kt2, 128)],
                        QT[:, h, ts(t, 512)],
                        start=True,
                        stop=True,
                    )
                    nc.scalar.activation(PT[:, kt2, :], ps[:], Exp, scale=scale)

            def do_pv(t, h):
                PT = pts.pop((t, h))
                for j in range(4):
                    tt = t * 4 + j
                    po = psov.tile([128, 130], F32, tag="po")
                    for kt2 in range(LT):
                        nc.tensor.matmul(
                            po[:, 0:129],
                            PT[:, kt2, ts(j, 128)],
                            V[:, kt2, h, 0:129],
                            start=(kt2 == 0),
                            stop=(kt2 == LT - 1),
                        )
                    rc = small.tile([128, 1], F32, tag="recip")
                    nc.vector.reciprocal(rc[:], po[:, 128:129])
                    nc.vector.tensor_scalar_mul(
                        O[:, tt, ts(h, 128)], po[:, 0:128], rc[:]
                    )

            def do_tail(t):
                # transpose this token group's O, then project it
                for j in range(4):
                    tt = t * 4 + j
                    for h in range(hpc):
                        tp = pstr.tile([128, 128], BF16, tag="tr")
                        nc.tensor.transpose(tp[:], O[:, tt, ts(h, 128)], ident[:])
                        nc.vector.tensor_copy(OT[:, h, ts(tt, 128)], tp[:])
                for j in range(4):
                    tt = t * 4 + j
                    for dc in range(dim // 512):
                        ps = psmm.tile([128, 512], F32, tag="mm512")
                        for h in range(hpc):
                            nc.tensor.matmul(
                                ps[:],
                                OT[:, h, ts(tt, 128)],
                                Wp_sb[:, h, ts(dc, 512)],
                                start=(h == 0),
                                stop=(h == hpc - 1),
                            )
                        ob = small.tile([128, 512], F32, tag="ob")
                        nc.vector.tensor_copy(ob[:], ps[:])
                        nc.sync.dma_start(out[ts(tt, 128), ts(dc, 512)], ob[:])

            # PV lags S^T/exp by 2 iterations so ACT's exp never gates PE
            LAG = 2
            n = len(iters)
            for i in range(n + LAG):
                if i < n:
                    do_st(*iters[i])
                if i >= LAG:
                    do_pv(*iters[i - LAG])
                    tp_, hp_ = iters[i - LAG]
                    if hp_ == hpc - 1:
                        do_tail(tp_)


def make_in_maps(x, context, W_qkv, b_qkv, W_proj):
    """Shard + pre-layout full inputs into per-core input maps."""
    x = np.asarray(x, dtype=np.float32)
    context = np.asarray(context, dtype=np.float32)
    W_qkv = np.asarray(W_qkv, dtype=np.float32)
    b_qkv = np.asarray(b_qkv, dtype=np.float32)
    W_proj = np.asarray(W_proj, dtype=np.float32)

    in_maps = []
    for c in range(N_CORES):
        b, g = divmod(c, GPB)
        n0 = g * NPC
        xTb = np.ascontiguousarray(x[b].T).astype(BF16_NP)
        cTb = np.ascontiguousarray(context[b].T).astype(BF16_NP)
        in_maps.append(
            {
                "xT": xTb,
                "cT": cTb,
                "wq": np.ascontiguousarray(W_qkv[:, n0 : n0 + NPC]).astype(BF16_NP),
                "wk": np.ascontiguousarray(
                    W_qkv[:, DIM + n0 : DIM + n0 + NPC]
                ).astype(BF16_NP),
                "wv": np.ascontiguousarray(
                    W_qkv[:, 2 * DIM + n0 : 2 * DIM + n0 + NPC]
                ).astype(BF16_NP),
                "bq": np.ascontiguousarray(
                    b_qkv[n0 : n0 + NPC].reshape(HPC, 128).T
                ).astype(np.float32),
                "bk": np.ascontiguousarray(
                    b_qkv[DIM + n0 : DIM + n0 + NPC].reshape(HPC, 128).T
                ).astype(np.float32),
                "bv": np.ascontiguousarray(
                    b_qkv[2 * DIM + n0 : 2 * DIM + n0 + NPC].reshape(1, NPC)
                ).astype(BF16_NP),
                "wp": np.ascontiguousarray(W_proj[n0 : n0 + NPC, :]).astype(BF16_NP),
            }
        )
    return in_maps


_NC_CACHE = {}


def kernel(x, context, W_qkv, b_qkv, W_proj, b_proj, _trace=False):
    from concourse.bass_utils import run_bass_kernel_spmd

    b_proj = np.asarray(b_proj, dtype=np.float32)
    in_maps = make_in_maps(x, context, W_qkv, b_qkv, W_proj)

    if "nc" not in _NC_CACHE:
        _NC_CACHE["nc"] = build_bass()
    nc = _NC_CACHE["nc"]

    res = run_bass_kernel_spmd(nc, in_maps, list(range(N_CORES)), trace=_trace)
    results = res.results

    out = np.zeros((B, L, DIM), dtype=np.float32)
    for c in range(N_CORES):
        b = c // GPB
        out[b] += results[c]["out"]
    out += b_proj[None, None, :]
    if _trace:
        return out, res
    return out

